# revision 55
# baseline (speedup 1.0000x reference)
"""Trainium2 Bass kernel for nn_AttentionProbe_80891414053184.

Math (reference):
    y  = relu(x @ W1.T + b1)            # (B,S,H) -> (B,S,128)
    y2 = relu(y @ W2.T + b2)            # (B,S,128)
    l  = y2 @ Wq.T + pos*pos_w  (+mask) # (B,S,8) logits
    p  = softmax(l, axis=S)
    v  = y2 @ Wv.T + bv
    out[b] = sum_{s,h} p*v + bias       # (B,1)

Strategy: sequence-parallel over 8 cores (512 positions x 4 batches = 2048
tokens per core).  The kernel is HBM-bound: the fp8 x-shard (8.4 MB)
streams at the ~358 GB/s per-core cap; everything else hides under it.

- x streams K-MAJOR (each pair-group of contraction rows covers all 2048
  tokens): consecutive matmuls share the same stationary weights, so the
  256-row DoubleRow LDWEIGHTS amortizes over 4 matmuls and stays off the
  critical path.  (Tile-major and half-split orders were measured and
  lose 2-12 us: fresh weight loads serialize with their matmuls, and more
  smaller DMAs slow the stream ramp.)
- Layer 1 is DoubleRow fp8 (ifmap AP [p, 2, N], pair step %16==0 -- the
  BIR-verifier-blessed form; ~216 ns per matmul when the PE is warm).
- All x DMAs sit on the SP HWDGE queue in consumption order (small first
  and last groups shorten the ramp and the end backlog); the last two
  pairs arrive per token tile so the 4 MLP tail chains stagger into the
  stream's tail.  Consts go on the gpsimd (SWDGE) queue in parallel.
  Every DMA source is per-partition contiguous (128 descriptors).
- MLP tail runs in bf16 (y, y2, W2/64, Wq, Wv): fewer const bytes, 2x
  16-bit DVE rate.  End-to-end error ~6e-3 vs the 2e-2 gate.
- Softmax without an on-chip max pass: the host folds
  c_h = max(0, pos_w_h*(S-1)) into the additive logit table (the
  remaining y2@Wq part is O(10), safe in fp32 exp), added into the q psum
  via a small identity matmul off the critical path.  bv folds into the
  host merge: sum p*(v+bv) = W/Z + bv.  The exp/mul/reduce stats chain is
  split into column halves so ACT and DVE pipeline.

NOTE: tensor_tensor_reduce hangs the device (NRT_EXEC_UNIT_UNRECOVERABLE);
use separate tensor_mul + tensor_reduce.
"""

import os

import numpy as np

# Problem dims (hardcoded per harness contract).
B, S, H = 4, 4096, 4096
MLP, NH = 128, 8
NCORES = 8
S_SHARD = S // NCORES        # 512 seq positions per core
TOK = B * S_SHARD            # 2048 tokens per core
NT = TOK // 512              # 4 token tiles of 512 (= one batch each)
KCH = H // 128               # 32 contraction chunks of 128
NPAIR = KCH // 2             # 16 DoubleRow pairs
NTAILP = 1                   # pairs delivered per-token-tile at the end:
# with a single tail pair, its weight load is shared across the 4 tile
# matmuls AND each matmul gates only on its own tile's 128 KB transfer,
# so the tile stops stagger with DMA arrival (~0.36 us apart).
NFULLP = NPAIR - NTAILP      # 15 pairs delivered full-width
# full-group sizes in pairs: small first groups (fast ramp) and small last
# groups (small end backlog)
GSCHED = [1, 1, 2, 2, 2, 2, 2, 2, 1]
assert sum(GSCHED) == NFULLP
P32 = NT * NH                # 32 packed (tile, head) lanes

_cache = {}


def _build_nc():
    import concourse.mybir as mybir
    import concourse.tile as tile
    from concourse import bacc
    from concourse.tile import add_dep_helper

    f32 = mybir.dt.float32
    f32r = mybir.dt.float32r
    bf16 = mybir.dt.bfloat16
    fp8 = mybir.dt.float8e4

    nc = bacc.Bacc()
    # xt[p, j, i, n] = x_shardT[128*(2j+i)+p, n]
    xt_d = nc.dram_tensor("xt", [128, NFULLP, 2, TOK], fp8,
                          kind="ExternalInput")
    # tail pairs, per token tile: xtl[p, t, j, i, n]
    xtl_d = nc.dram_tensor("xtl", [128, NT, NTAILP, 2, 512], fp8,
                           kind="ExternalInput")
    w1_d = nc.dram_tensor("w1s", [128, KCH, MLP], fp8, kind="ExternalInput")
    # cwb (bf16): [w2t/64 | wq32 (4 x 32-wide zero-padded blocks) | wv32]
    CQ = MLP
    CV = MLP + P32 * NT
    CWBW = MLP + 2 * P32 * NT
    cwb_d = nc.dram_tensor("cwb", [MLP, CWBW], bf16, kind="ExternalInput")
    ci_d = nc.dram_tensor("ci", [P32, P32], f32r, kind="ExternalInput")
    cb_d = nc.dram_tensor("cb", [MLP, 2], f32, kind="ExternalInput")  # 64b1|b2
    # ca row 8t+h = (batch tile t, head h): pos*pos_w - c_h + mask add
    ca_d = nc.dram_tensor("ca", [P32, 512], f32r, kind="ExternalInput")
    # stats: [Z_lo, W_lo, Z_hi, W_hi]
    st_d = nc.dram_tensor("stats", [P32, 4], f32, kind="ExternalOutput")

    AF = mybir.ActivationFunctionType
    AX = mybir.AxisListType
    OP = mybir.AluOpType
    PM = mybir.MatmulPerfMode.DoubleRow

    with tile.TileContext(nc) as tc:
        with (
            tc.tile_pool(name="const", bufs=1) as const,
            tc.tile_pool(name="xp", bufs=len(GSCHED)) as xp,
            tc.tile_pool(name="xlp", bufs=NT) as xlp,
            tc.tile_pool(name="yp", bufs=4) as yp,
            tc.tile_pool(name="y2p", bufs=4) as y2p,
            tc.tile_pool(name="smallp", bufs=1) as smallp,
            tc.tile_pool(name="statsp", bufs=1) as statsp,
            tc.tile_pool(name="ps_y", bufs=4, space="PSUM") as ps_y,
            tc.tile_pool(name="ps_w", bufs=1, space="PSUM") as ps_w,
            tc.tile_pool(name="ps_q", bufs=1, space="PSUM") as ps_q,
            tc.tile_pool(name="ps_v", bufs=1, space="PSUM") as ps_v,
        ):
            w1_sb = const.tile([128, KCH, MLP], fp8)
            cwb_sb = const.tile([MLP, CWBW], bf16)
            ci_sb = const.tile([P32, P32], f32r)
            cb_sb = const.tile([MLP, 2], f32)
            ca_sb = const.tile([P32, 512], f32r)

            x_sb = []
            pbase = []
            p0 = 0
            for gi, gsz in enumerate(GSCHED):
                xg = xp.tile([128, gsz, 2, TOK], fp8, tag="x", name=f"x{gi}")
                x_sb.append(xg)
                pbase.append(p0)
                p0 += gsz
            xl_sb = [xlp.tile([128, NTAILP, 2, 512], fp8, tag="xl",
                              name=f"xl{t}") for t in range(NT)]

            # sync queue: ALL x transfers in consumption order.
            # gpsimd queue: the small consts, in parallel.
            nc.gpsimd.dma_start(out=w1_sb[:, 0:4, :], in_=w1_d[:, 0:4, :])
            for gi, gsz in enumerate(GSCHED):
                nc.sync.dma_start(
                    out=x_sb[gi][:],
                    in_=xt_d[:, pbase[gi]:pbase[gi] + gsz, :, :])
                if gi == 0:
                    nc.gpsimd.dma_start(out=w1_sb[:, 4:KCH, :],
                                        in_=w1_d[:, 4:KCH, :])
                    nc.gpsimd.dma_start(out=cwb_sb[:], in_=cwb_d[:])
                    nc.gpsimd.dma_start(out=ci_sb[:], in_=ci_d[:])
                    nc.gpsimd.dma_start(out=cb_sb[:], in_=cb_d[:])
                    nc.gpsimd.dma_start(out=ca_sb[:], in_=ca_d[:])
            for t in range(NT):
                nc.sync.dma_start(out=xl_sb[t][:], in_=xtl_d[:, t, :, :, :])

            stats_sb = statsp.tile([P32, 4], f32)

            # HAM warm-up: the PE sits idle through the ~12 us preamble +
            # first-DMA ramp, so HAM throttles its clock to 1.2 GHz and the
            # first ~10 real matmuls run at half speed (deferring ~2 us of
            # PE work past the stream's end).  Chew through dummy matmuls on
            # a memset tile during the ramp so the real stream starts warm.
            warm_ps = ps_w.tile([128, 512], f32, tag="warm", name="warm_ps")
            dw_sb = const.tile([128, 256], bf16)
            nc.vector.memset(dw_sb[:], 0.0)

            def pe_dummies(n):
                for _ in range(n):
                    nc.tensor.matmul(warm_ps[:, 0:256], dw_sb[:, 0:128],
                                     dw_sb[:, 0:256], start=True, stop=True)

            pe_dummies(22)

            # Warmup: PE observes the w1 first-group DMA lane before the real
            # matmuls so steady-state instructions carry at most one new wait.
            warm_pe = nc.tensor.matmul(warm_ps[0:32, 0:64],
                                       w1_sb[:, 0, 0:32], w1_sb[:, 0, 0:64],
                                       start=True, stop=True)
            warm_act = const.tile([MLP, 1], f32)
            nc.scalar.copy(out=warm_act[:], in_=cb_sb[:, 1:2])
            warm_dve = const.tile([MLP, 1], f32)
            nc.vector.tensor_copy(out=warm_dve[:], in_=cb_sb[:, 0:1])

            # Layer 1, k-major: yT[t] (128, 512) += (64*W1T)_pair.T @ x_pair,
            # DoubleRow fp8, accumulated over the 16 pairs.
            psum_y = [ps_y.tile([128, 512], f32, tag="y", name=f"y_ps{t}")
                      for t in range(NT)]
            for gi, gsz in enumerate(GSCHED):
                for jj in range(gsz):
                    jp = pbase[gi] + jj
                    for t in range(NT):
                        mm = nc.tensor.matmul(
                            psum_y[t][:],
                            w1_sb[:, 2 * jp:2 * jp + 2, :],
                            x_sb[gi][:, jj, :, t * 512:(t + 1) * 512],
                            start=(jp == 0), stop=False,
                            perf_mode=PM)
                        if jp == 0 and t == 0:
                            add_dep_helper(mm.ins, warm_pe.ins, sync=False,
                                           reason="warmup before first mm")
                # The early DMA-paced groups leave the PE ~50% idle, so HAM
                # holds the clock at 1.2 GHz and the deficit is repaid after
                # the stream ends.  Fill the waits with dummies to keep
                # utilization high until the stream itself saturates the PE.
                if gi < 4:
                    pe_dummies(5 if gi < 2 else 4)
            # per-tile tail pairs: arrival order staggers the tile stops so
            # the DVE relu cascade starts as each tile's data lands
            for t in range(NT):
                for j in range(NTAILP):
                    jp = NFULLP + j
                    nc.tensor.matmul(psum_y[t][:],
                                     w1_sb[:, 2 * jp:2 * jp + 2, :],
                                     xl_sb[t][:, j, :, :],
                                     start=False, stop=(jp == NPAIR - 1),
                                     perf_mode=PM)

            q32_ps = ps_q.tile([P32, 512], f32, tag="q", name="q32_ps")
            v32_ps = ps_v.tile([P32, 512], f32, tag="v", name="v32_ps")
            # Fold the additive logit table into the q psum via an identity
            # matmul -- runs as soon as ca lands, off the critical tail path.
            nc.tensor.matmul(q32_ps[:], ci_sb[:], ca_sb[:],
                             start=True, stop=False)
            # Two-phase tail with DVE/ACT braiding: phase A drains each y
            # psum through relu1 (alternating engines) into its W2 matmul --
            # the y2 psum REUSES the y bank its relu1 just freed.  Phase B
            # runs the relu2s (alternating the other way) and the head
            # projections.  Emitting all relu1s before all relu2s keeps the
            # strict-FIFO ACT/DVE queues free of head-of-line blocking.
            y2_pss = []
            y2_sbs = []
            for t in range(NT):
                y_sb = yp.tile([128, 512], bf16, tag="ysb", name=f"y_sb{t}")
                # y_sb = relu(psum + 64*b1) = 64*y; the 1/64 is folded into
                # cwb's W2 block.
                if t % 2 == 0:
                    nc.vector.tensor_scalar(out=y_sb[:], in0=psum_y[t][:],
                                            scalar1=cb_sb[:, 0:1],
                                            scalar2=0.0, op0=OP.add,
                                            op1=OP.max)
                else:
                    nc.scalar.activation(out=y_sb[:], in_=psum_y[t][:],
                                         func=AF.Relu, bias=cb_sb[:, 0:1],
                                         scale=1.0)
                y2_ps = ps_y.tile([128, 512], f32, tag="y",
                                  name=f"y2_ps{t}")
                nc.tensor.matmul(y2_ps[:], cwb_sb[:, 0:MLP], y_sb[:],
                                 start=True, stop=True)
                y2_pss.append(y2_ps)
            for t in range(NT):
                y2_sb = y2p.tile([128, 512], bf16, tag="y2sb",
                                 name=f"y2_sb{t}")
                if t % 2 == 0:
                    nc.scalar.activation(out=y2_sb[:], in_=y2_pss[t][:],
                                         func=AF.Relu, bias=cb_sb[:, 1:2],
                                         scale=1.0)
                else:
                    nc.vector.tensor_scalar(out=y2_sb[:], in0=y2_pss[t][:],
                                            scalar1=cb_sb[:, 1:2],
                                            scalar2=0.0, op0=OP.add,
                                            op1=OP.max)
                y2_sbs.append(y2_sb)
                # Head projections: the (128, 32) weight block for tile t is
                # zero outside rows 8t..8t+8, so accumulating all 4 tiles into
                # one (32, 512) bank packs q/v as (tile, head) x seq lanes.
                nc.tensor.matmul(q32_ps[:],
                                 cwb_sb[:, CQ + P32 * t:CQ + P32 * (t + 1)],
                                 y2_sb[:], start=False, stop=(t == NT - 1))
                nc.tensor.matmul(v32_ps[:],
                                 cwb_sb[:, CV + P32 * t:CV + P32 * (t + 1)],
                                 y2_sb[:], start=(t == 0), stop=(t == NT - 1))

            # Stats, split into column halves so ACT (exp) and DVE
            # (mul/reduce) pipeline: Z = sum e, W = sum e*v per half.
            e_sb = smallp.tile([P32, 512], f32, tag="e", name="e_sb")
            ev_sb = smallp.tile([P32, 512], f32, tag="ev", name="ev_sb")
            for half in range(2):
                cols = slice(256 * half, 256 * (half + 1))
                nc.scalar.activation(out=e_sb[:, cols], in_=q32_ps[:, cols],
                                     func=AF.Exp, bias=0.0, scale=1.0,
                                     accum_out=stats_sb[:, 2 * half:
                                                        2 * half + 1])
                nc.vector.tensor_mul(out=ev_sb[:, cols], in0=e_sb[:, cols],
                                     in1=v32_ps[:, cols])
                nc.vector.tensor_reduce(
                    out=stats_sb[:, 2 * half + 1:2 * half + 2],
                    in_=ev_sb[:, cols], axis=AX.X, op=OP.add)

            nc.sync.dma_start(out=st_d[:], in_=stats_sb[:])

    nc.finalize()
    return nc


def get_nc():
    if "nc" not in _cache:
        _cache["nc"] = _build_nc()
    return _cache["nc"]


def make_core_inputs(x, mask, W1, b1, W2, b2, Wq, Wv, bv, pos_w, bias):
    """Host-side shard + transpose + fp8 quantize. Returns 8 in_maps."""
    import ml_dtypes
    FP8 = ml_dtypes.float8_e4m3
    BF16 = ml_dtypes.bfloat16

    # W1 scaled by 64 so its values quantize in e4m3's normal range; the
    # matching 1/64 is folded into W2 below (exact: power of two).
    w1s = np.ascontiguousarray(
        (W1.astype(np.float32) * 64.0).reshape(MLP, KCH, 128)
        .transpose(2, 1, 0)).astype(FP8)

    CQ = MLP
    CV = MLP + P32 * NT
    cwb = np.zeros((MLP, MLP + 2 * P32 * NT), dtype=np.float32)
    cwb[:, 0:MLP] = W2.T / 64.0
    for t in range(NT):
        cwb[:, CQ + P32 * t + NH * t:CQ + P32 * t + NH * (t + 1)] = Wq.T
        cwb[:, CV + P32 * t + NH * t:CV + P32 * t + NH * (t + 1)] = Wv.T
    cwb = cwb.astype(BF16)
    ci = np.eye(P32, dtype=np.float32)
    cb = np.ascontiguousarray(
        np.stack([b1.astype(np.float32) * 64.0,
                  b2.astype(np.float32)], axis=1), dtype=np.float32)

    pos = np.arange(S, dtype=np.float32)
    maskadd = np.where(mask == 0, np.float32(-1e9), np.float32(0.0))  # (B,S)
    # Host-side stability offset: dominant logit term over the FULL sequence.
    c_h = np.maximum(pos_w.astype(np.float32) * (S - 1), 0.0)       # (NH,)

    in_maps = []
    for c in range(NCORES):
        sl = slice(c * S_SHARD, (c + 1) * S_SHARD)
        xT = np.ascontiguousarray(
            x[:, sl, :].transpose(2, 0, 1).reshape(H, TOK)).astype(FP8)
        xr = xT.reshape(NPAIR, 2, 128, TOK)                 # (jp, i, p, n)
        xt8 = np.ascontiguousarray(xr[0:NFULLP].transpose(2, 0, 1, 3))
        xtl = np.ascontiguousarray(
            xr[NFULLP:].reshape(NTAILP, 2, 128, NT, 512)
            .transpose(2, 3, 0, 1, 4))              # (128, NT, 2, 2, 512)
        ca = np.empty((P32, 512), dtype=np.float32)
        add_ths = (pos_w.astype(np.float32)[None, :, None]
                   * pos[sl][None, None, :]
                   - c_h[None, :, None]
                   + maskadd[:, None, sl])            # (B=NT, NH, 512)
        ca[:, :] = add_ths.reshape(P32, 512)
        in_maps.append({"xt": xt8, "xtl": xtl, "w1s": w1s, "cwb": cwb,
                        "ci": ci, "cb": cb, "ca": ca})
    return in_maps


def merge_stats(stats_all, bv, bias):
    """stats_all: (NCORES, 32, 4), row 8t+h = (batch t, head h) with
    [Z_lo, W_lo, Z_hi, W_hi].  All cores share the same per-head logit
    offset, so the merge is a plain sum.  bv folds in on the host:
    sum_s p*(v+bv) = W/Z + bv."""
    st = np.asarray(stats_all, dtype=np.float64).reshape(NCORES, NT, NH, 4)
    Z = (st[..., 0] + st[..., 2]).sum(axis=0)        # (B, NH)
    W = (st[..., 1] + st[..., 3]).sum(axis=0)
    out = (W / Z + np.asarray(bv, dtype=np.float64)[None, :]).sum(axis=1)
    return (out[:, None] + np.float64(bias.reshape(1)[0])).astype(np.float32)


def kernel(x, mask, W1, b1, W2, b2, Wq, Wv, bv, pos_w, bias, _trace=False):
    from concourse.bass_utils import run_bass_kernel_spmd

    x = np.asarray(x, dtype=np.float32)
    in_maps = make_core_inputs(x, np.asarray(mask), *(np.asarray(a) for a in
                               (W1, b1, W2, b2, Wq, Wv, bv, pos_w, bias)))
    nc = get_nc()
    res = run_bass_kernel_spmd(nc, in_maps, core_ids=list(range(NCORES)),
                               trace=_trace)
    stats_all = np.stack([r["stats"] for r in res.results])  # (C, 32, 4)
    out = merge_stats(stats_all, np.asarray(bv), np.asarray(bias))
    if _trace:
        kernel.last_result = res
    return out


# revision 57
# speedup vs baseline: 1.1013x; 1.1013x over previous
"""Trainium2 Bass kernel for nn_AttentionProbe_80891414053184.

Math (reference):
    y  = relu(x @ W1.T + b1)            # (B,S,H) -> (B,S,128)
    y2 = relu(y @ W2.T + b2)            # (B,S,128)
    l  = y2 @ Wq.T + pos*pos_w  (+mask) # (B,S,8) logits
    p  = softmax(l, axis=S)
    v  = y2 @ Wv.T + bv
    out[b] = sum_{s,h} p*v + bias       # (B,1)

Strategy: sequence-parallel over 8 cores (512 positions x 4 batches = 2048
tokens per core).  The kernel is HBM-bound: the fp8 x-shard (8.4 MB)
streams at the ~358 GB/s per-core cap; everything else hides under it.

- x streams K-MAJOR (each pair-group of contraction rows covers all 2048
  tokens): consecutive matmuls share the same stationary weights, so the
  256-row DoubleRow LDWEIGHTS amortizes over 4 matmuls and stays off the
  critical path.  (Tile-major and half-split orders were measured and
  lose 2-12 us: fresh weight loads serialize with their matmuls, and more
  smaller DMAs slow the stream ramp.)
- Layer 1 is DoubleRow fp8 (ifmap AP [p, 2, N], pair step %16==0 -- the
  BIR-verifier-blessed form; ~216 ns per matmul when the PE is warm).
- All x DMAs sit on the SP HWDGE queue in consumption order (small first
  and last groups shorten the ramp and the end backlog); the last two
  pairs arrive per token tile so the 4 MLP tail chains stagger into the
  stream's tail.  Consts go on the gpsimd (SWDGE) queue in parallel.
  Every DMA source is per-partition contiguous (128 descriptors).
- MLP tail runs in bf16 (y, y2, W2/64, Wq, Wv): fewer const bytes, 2x
  16-bit DVE rate.  End-to-end error ~6e-3 vs the 2e-2 gate.
- Softmax without an on-chip max pass: the host folds
  c_h = max(0, pos_w_h*(S-1)) into the additive logit table (the
  remaining y2@Wq part is O(10), safe in fp32 exp), added into the q psum
  via a small identity matmul off the critical path.  bv folds into the
  host merge: sum p*(v+bv) = W/Z + bv.  The exp/mul/reduce stats chain is
  split into column halves so ACT and DVE pipeline.

NOTE: tensor_tensor_reduce hangs the device (NRT_EXEC_UNIT_UNRECOVERABLE);
use separate tensor_mul + tensor_reduce.
"""

import os

import numpy as np

# Problem dims (hardcoded per harness contract).
B, S, H = 4, 4096, 4096
MLP, NH = 128, 8
NCORES = 8
S_SHARD = S // NCORES        # 512 seq positions per core
TOK = B * S_SHARD            # 2048 tokens per core
NT = TOK // 512              # 4 token tiles of 512 (= one batch each)
KCH = H // 128               # 32 contraction chunks of 128
NPAIR = KCH // 2             # 16 DoubleRow pairs
NTAILP = 1                   # pairs delivered per-token-tile at the end:
# with a single tail pair, its weight load is shared across the 4 tile
# matmuls AND each matmul gates only on its own tile's 128 KB transfer,
# so the tile stops stagger with DMA arrival (~0.36 us apart).
NFULLP = NPAIR - NTAILP      # 15 pairs delivered full-width
# full-group sizes in pairs: small first groups (fast ramp) and small last
# groups (small end backlog)
GSCHED = [1, 1, 2, 2, 2, 2, 2, 2, 1]
assert sum(GSCHED) == NFULLP
P32 = NT * NH                # 32 packed (tile, head) lanes

_cache = {}


def _build_nc():
    import concourse.mybir as mybir
    import concourse.tile as tile
    from concourse import bacc
    from concourse.tile import add_dep_helper

    f32 = mybir.dt.float32
    f32r = mybir.dt.float32r
    bf16 = mybir.dt.bfloat16
    fp8 = mybir.dt.float8e4

    nc = bacc.Bacc()
    # xt[p, j, i, n] = x_shardT[128*(2j+i)+p, n]
    xt_d = nc.dram_tensor("xt", [128, NFULLP, 2, TOK], fp8,
                          kind="ExternalInput")
    # tail pairs, per token tile: xtl[p, t, j, i, n]
    xtl_d = nc.dram_tensor("xtl", [128, NT, NTAILP, 2, 512], fp8,
                           kind="ExternalInput")
    w1_d = nc.dram_tensor("w1s", [128, KCH, MLP], fp8, kind="ExternalInput")
    # cwb (bf16): [w2t/64 | wq32 (4 x 32-wide zero-padded blocks) | wv32]
    CQ = MLP
    CV = MLP + P32 * NT
    CWBW = MLP + 2 * P32 * NT
    cwb_d = nc.dram_tensor("cwb", [MLP, CWBW], bf16, kind="ExternalInput")
    ci_d = nc.dram_tensor("ci", [P32, P32], f32r, kind="ExternalInput")
    cb_d = nc.dram_tensor("cb", [MLP, 2], f32, kind="ExternalInput")  # 64b1|b2
    # ca row 8t+h = (batch tile t, head h): pos*pos_w - c_h + mask add
    ca_d = nc.dram_tensor("ca", [P32, 512], f32r, kind="ExternalInput")
    # stats: [Z_lo, W_lo, Z_hi, W_hi]
    st_d = nc.dram_tensor("stats", [P32, 4], f32, kind="ExternalOutput")

    AF = mybir.ActivationFunctionType
    AX = mybir.AxisListType
    OP = mybir.AluOpType
    PM = mybir.MatmulPerfMode.DoubleRow

    with tile.TileContext(nc) as tc:
        with (
            tc.tile_pool(name="const", bufs=1) as const,
            tc.tile_pool(name="xp", bufs=len(GSCHED)) as xp,
            tc.tile_pool(name="xlp", bufs=NT) as xlp,
            tc.tile_pool(name="yp", bufs=4) as yp,
            tc.tile_pool(name="y2p", bufs=4) as y2p,
            tc.tile_pool(name="smallp", bufs=1) as smallp,
            tc.tile_pool(name="statsp", bufs=1) as statsp,
            tc.tile_pool(name="ps_y", bufs=4, space="PSUM") as ps_y,
            tc.tile_pool(name="ps_w", bufs=1, space="PSUM") as ps_w,
            tc.tile_pool(name="ps_q", bufs=1, space="PSUM") as ps_q,
            tc.tile_pool(name="ps_v", bufs=1, space="PSUM") as ps_v,
        ):
            w1_sb = const.tile([128, KCH, MLP], fp8)
            cwb_sb = const.tile([MLP, CWBW], bf16)
            ci_sb = const.tile([P32, P32], f32r)
            cb_sb = const.tile([MLP, 2], f32)
            ca_sb = const.tile([P32, 512], f32r)

            x_sb = []
            pbase = []
            p0 = 0
            for gi, gsz in enumerate(GSCHED):
                xg = xp.tile([128, gsz, 2, TOK], fp8, tag="x", name=f"x{gi}")
                x_sb.append(xg)
                pbase.append(p0)
                p0 += gsz
            xl_sb = [xlp.tile([128, NTAILP, 2, 512], fp8, tag="xl",
                              name=f"xl{t}") for t in range(NT)]

            # sync queue: ALL x transfers in consumption order.
            # gpsimd queue: the small consts, in parallel.
            nc.gpsimd.dma_start(out=w1_sb[:, 0:4, :], in_=w1_d[:, 0:4, :])
            for gi, gsz in enumerate(GSCHED):
                nc.sync.dma_start(
                    out=x_sb[gi][:],
                    in_=xt_d[:, pbase[gi]:pbase[gi] + gsz, :, :])
                if gi == 0:
                    nc.gpsimd.dma_start(out=w1_sb[:, 4:KCH, :],
                                        in_=w1_d[:, 4:KCH, :])
                    nc.gpsimd.dma_start(out=cwb_sb[:], in_=cwb_d[:])
                    nc.gpsimd.dma_start(out=ci_sb[:], in_=ci_d[:])
                    nc.gpsimd.dma_start(out=cb_sb[:], in_=cb_d[:])
                    nc.gpsimd.dma_start(out=ca_sb[:], in_=ca_d[:])
            for t in range(NT):
                nc.sync.dma_start(out=xl_sb[t][:], in_=xtl_d[:, t, :, :, :])

            stats_sb = statsp.tile([P32, 4], f32)

            # HAM warm-up: the PE sits idle through the ~12 us preamble +
            # first-DMA ramp, so HAM throttles its clock to 1.2 GHz and the
            # first ~10 real matmuls run at half speed (deferring ~2 us of
            # PE work past the stream's end).  Chew through dummy matmuls on
            # a memset tile during the ramp so the real stream starts warm.
            warm_ps = ps_w.tile([128, 512], f32, tag="warm", name="warm_ps")
            dw_sb = const.tile([128, 256], bf16)
            nc.vector.memset(dw_sb[:], 0.0)

            def pe_dummies(n):
                for _ in range(n):
                    nc.tensor.matmul(warm_ps[:, 0:256], dw_sb[:, 0:128],
                                     dw_sb[:, 0:256], start=True, stop=True)

            pe_dummies(16)

            # Warmup: PE observes the w1 first-group DMA lane before the real
            # matmuls so steady-state instructions carry at most one new wait.
            warm_pe = nc.tensor.matmul(warm_ps[0:32, 0:64],
                                       w1_sb[:, 0, 0:32], w1_sb[:, 0, 0:64],
                                       start=True, stop=True)
            warm_act = const.tile([MLP, 1], f32)
            nc.scalar.copy(out=warm_act[:], in_=cb_sb[:, 1:2])
            warm_dve = const.tile([MLP, 1], f32)
            nc.vector.tensor_copy(out=warm_dve[:], in_=cb_sb[:, 0:1])

            # Layer 1, k-major: yT[t] (128, 512) += (64*W1T)_pair.T @ x_pair,
            # DoubleRow fp8, accumulated over the 16 pairs.
            psum_y = [ps_y.tile([128, 512], f32, tag="y", name=f"y_ps{t}")
                      for t in range(NT)]
            for gi, gsz in enumerate(GSCHED):
                for jj in range(gsz):
                    jp = pbase[gi] + jj
                    for t in range(NT):
                        mm = nc.tensor.matmul(
                            psum_y[t][:],
                            w1_sb[:, 2 * jp:2 * jp + 2, :],
                            x_sb[gi][:, jj, :, t * 512:(t + 1) * 512],
                            start=(jp == 0), stop=False,
                            perf_mode=PM)
                        if jp == 0 and t == 0:
                            add_dep_helper(mm.ins, warm_pe.ins, sync=False,
                                           reason="warmup before first mm")

            # per-tile tail pairs: arrival order staggers the tile stops so
            # the DVE relu cascade starts as each tile's data lands
            for t in range(NT):
                for j in range(NTAILP):
                    jp = NFULLP + j
                    nc.tensor.matmul(psum_y[t][:],
                                     w1_sb[:, 2 * jp:2 * jp + 2, :],
                                     xl_sb[t][:, j, :, :],
                                     start=False, stop=(jp == NPAIR - 1),
                                     perf_mode=PM)

            q32_ps = ps_q.tile([P32, 512], f32, tag="q", name="q32_ps")
            v32_ps = ps_v.tile([P32, 512], f32, tag="v", name="v32_ps")
            # Fold the additive logit table into the q psum via an identity
            # matmul -- runs as soon as ca lands, off the critical tail path.
            nc.tensor.matmul(q32_ps[:], ci_sb[:], ca_sb[:],
                             start=True, stop=False)
            # Two-phase tail with DVE/ACT braiding: phase A drains each y
            # psum through relu1 (alternating engines) into its W2 matmul --
            # the y2 psum REUSES the y bank its relu1 just freed.  Phase B
            # runs the relu2s (alternating the other way) and the head
            # projections.  Emitting all relu1s before all relu2s keeps the
            # strict-FIFO ACT/DVE queues free of head-of-line blocking.
            y2_pss = []
            y2_sbs = []
            for t in range(NT):
                y_sb = yp.tile([128, 512], bf16, tag="ysb", name=f"y_sb{t}")
                # y_sb = relu(psum + 64*b1) = 64*y; the 1/64 is folded into
                # cwb's W2 block.
                if t % 2 == 0:
                    nc.vector.tensor_scalar(out=y_sb[:], in0=psum_y[t][:],
                                            scalar1=cb_sb[:, 0:1],
                                            scalar2=0.0, op0=OP.add,
                                            op1=OP.max)
                else:
                    nc.scalar.activation(out=y_sb[:], in_=psum_y[t][:],
                                         func=AF.Relu, bias=cb_sb[:, 0:1],
                                         scale=1.0)
                y2_ps = ps_y.tile([128, 512], f32, tag="y",
                                  name=f"y2_ps{t}")
                nc.tensor.matmul(y2_ps[:], cwb_sb[:, 0:MLP], y_sb[:],
                                 start=True, stop=True)
                y2_pss.append(y2_ps)
            for t in range(NT):
                y2_sb = y2p.tile([128, 512], bf16, tag="y2sb",
                                 name=f"y2_sb{t}")
                if t % 2 == 0:
                    nc.scalar.activation(out=y2_sb[:], in_=y2_pss[t][:],
                                         func=AF.Relu, bias=cb_sb[:, 1:2],
                                         scale=1.0)
                else:
                    nc.vector.tensor_scalar(out=y2_sb[:], in0=y2_pss[t][:],
                                            scalar1=cb_sb[:, 1:2],
                                            scalar2=0.0, op0=OP.add,
                                            op1=OP.max)
                y2_sbs.append(y2_sb)
                # Head projections: the (128, 32) weight block for tile t is
                # zero outside rows 8t..8t+8, so accumulating all 4 tiles into
                # one (32, 512) bank packs q/v as (tile, head) x seq lanes.
                nc.tensor.matmul(q32_ps[:],
                                 cwb_sb[:, CQ + P32 * t:CQ + P32 * (t + 1)],
                                 y2_sb[:], start=False, stop=(t == NT - 1))
                nc.tensor.matmul(v32_ps[:],
                                 cwb_sb[:, CV + P32 * t:CV + P32 * (t + 1)],
                                 y2_sb[:], start=(t == 0), stop=(t == NT - 1))

            # Stats, split into column halves so ACT (exp) and DVE
            # (mul/reduce) pipeline: Z = sum e, W = sum e*v per half.
            e_sb = smallp.tile([P32, 512], f32, tag="e", name="e_sb")
            ev_sb = smallp.tile([P32, 512], f32, tag="ev", name="ev_sb")
            for half in range(2):
                cols = slice(256 * half, 256 * (half + 1))
                nc.scalar.activation(out=e_sb[:, cols], in_=q32_ps[:, cols],
                                     func=AF.Exp, bias=0.0, scale=1.0,
                                     accum_out=stats_sb[:, 2 * half:
                                                        2 * half + 1])
                nc.vector.tensor_mul(out=ev_sb[:, cols], in0=e_sb[:, cols],
                                     in1=v32_ps[:, cols])
                nc.vector.tensor_reduce(
                    out=stats_sb[:, 2 * half + 1:2 * half + 2],
                    in_=ev_sb[:, cols], axis=AX.X, op=OP.add)

            nc.sync.dma_start(out=st_d[:], in_=stats_sb[:])

    nc.finalize()
    return nc


def get_nc():
    if "nc" not in _cache:
        _cache["nc"] = _build_nc()
    return _cache["nc"]


def make_core_inputs(x, mask, W1, b1, W2, b2, Wq, Wv, bv, pos_w, bias):
    """Host-side shard + transpose + fp8 quantize. Returns 8 in_maps."""
    import ml_dtypes
    FP8 = ml_dtypes.float8_e4m3
    BF16 = ml_dtypes.bfloat16

    # W1 scaled by 64 so its values quantize in e4m3's normal range; the
    # matching 1/64 is folded into W2 below (exact: power of two).
    w1s = np.ascontiguousarray(
        (W1.astype(np.float32) * 64.0).reshape(MLP, KCH, 128)
        .transpose(2, 1, 0)).astype(FP8)

    CQ = MLP
    CV = MLP + P32 * NT
    cwb = np.zeros((MLP, MLP + 2 * P32 * NT), dtype=np.float32)
    cwb[:, 0:MLP] = W2.T / 64.0
    for t in range(NT):
        cwb[:, CQ + P32 * t + NH * t:CQ + P32 * t + NH * (t + 1)] = Wq.T
        cwb[:, CV + P32 * t + NH * t:CV + P32 * t + NH * (t + 1)] = Wv.T
    cwb = cwb.astype(BF16)
    ci = np.eye(P32, dtype=np.float32)
    cb = np.ascontiguousarray(
        np.stack([b1.astype(np.float32) * 64.0,
                  b2.astype(np.float32)], axis=1), dtype=np.float32)

    pos = np.arange(S, dtype=np.float32)
    maskadd = np.where(mask == 0, np.float32(-1e9), np.float32(0.0))  # (B,S)
    # Host-side stability offset: dominant logit term over the FULL sequence.
    c_h = np.maximum(pos_w.astype(np.float32) * (S - 1), 0.0)       # (NH,)

    in_maps = []
    for c in range(NCORES):
        sl = slice(c * S_SHARD, (c + 1) * S_SHARD)
        xT = np.ascontiguousarray(
            x[:, sl, :].transpose(2, 0, 1).reshape(H, TOK)).astype(FP8)
        xr = xT.reshape(NPAIR, 2, 128, TOK)                 # (jp, i, p, n)
        xt8 = np.ascontiguousarray(xr[0:NFULLP].transpose(2, 0, 1, 3))
        xtl = np.ascontiguousarray(
            xr[NFULLP:].reshape(NTAILP, 2, 128, NT, 512)
            .transpose(2, 3, 0, 1, 4))              # (128, NT, 2, 2, 512)
        ca = np.empty((P32, 512), dtype=np.float32)
        add_ths = (pos_w.astype(np.float32)[None, :, None]
                   * pos[sl][None, None, :]
                   - c_h[None, :, None]
                   + maskadd[:, None, sl])            # (B=NT, NH, 512)
        ca[:, :] = add_ths.reshape(P32, 512)
        in_maps.append({"xt": xt8, "xtl": xtl, "w1s": w1s, "cwb": cwb,
                        "ci": ci, "cb": cb, "ca": ca})
    return in_maps


def merge_stats(stats_all, bv, bias):
    """stats_all: (NCORES, 32, 4), row 8t+h = (batch t, head h) with
    [Z_lo, W_lo, Z_hi, W_hi].  All cores share the same per-head logit
    offset, so the merge is a plain sum.  bv folds in on the host:
    sum_s p*(v+bv) = W/Z + bv."""
    st = np.asarray(stats_all, dtype=np.float64).reshape(NCORES, NT, NH, 4)
    Z = (st[..., 0] + st[..., 2]).sum(axis=0)        # (B, NH)
    W = (st[..., 1] + st[..., 3]).sum(axis=0)
    out = (W / Z + np.asarray(bv, dtype=np.float64)[None, :]).sum(axis=1)
    return (out[:, None] + np.float64(bias.reshape(1)[0])).astype(np.float32)


def kernel(x, mask, W1, b1, W2, b2, Wq, Wv, bv, pos_w, bias, _trace=False):
    from concourse.bass_utils import run_bass_kernel_spmd

    x = np.asarray(x, dtype=np.float32)
    in_maps = make_core_inputs(x, np.asarray(mask), *(np.asarray(a) for a in
                               (W1, b1, W2, b2, Wq, Wv, bv, pos_w, bias)))
    nc = get_nc()
    res = run_bass_kernel_spmd(nc, in_maps, core_ids=list(range(NCORES)),
                               trace=_trace)
    stats_all = np.stack([r["stats"] for r in res.results])  # (C, 32, 4)
    out = merge_stats(stats_all, np.asarray(bv), np.asarray(bias))
    if _trace:
        kernel.last_result = res
    return out


# revision 58
# speedup vs baseline: 1.1345x; 1.0302x over previous
"""Trainium2 Bass kernel for nn_AttentionProbe_80891414053184.

Math (reference):
    y  = relu(x @ W1.T + b1)            # (B,S,H) -> (B,S,128)
    y2 = relu(y @ W2.T + b2)            # (B,S,128)
    l  = y2 @ Wq.T + pos*pos_w  (+mask) # (B,S,8) logits
    p  = softmax(l, axis=S)
    v  = y2 @ Wv.T + bv
    out[b] = sum_{s,h} p*v + bias       # (B,1)

Strategy: sequence-parallel over 8 cores (512 positions x 4 batches = 2048
tokens per core).  The kernel is HBM-bound: the fp8 x-shard (8.4 MB)
streams at the ~358 GB/s per-core cap; everything else hides under it.

- x streams K-MAJOR (each pair-group of contraction rows covers all 2048
  tokens): consecutive matmuls share the same stationary weights, so the
  256-row DoubleRow LDWEIGHTS amortizes over 4 matmuls and stays off the
  critical path.  (Tile-major and half-split orders were measured and
  lose 2-12 us: fresh weight loads serialize with their matmuls, and more
  smaller DMAs slow the stream ramp.)
- Layer 1 is DoubleRow fp8 (ifmap AP [p, 2, N], pair step %16==0 -- the
  BIR-verifier-blessed form; ~216 ns per matmul when the PE is warm).
- All x DMAs sit on the SP HWDGE queue in consumption order (small first
  and last groups shorten the ramp and the end backlog); the last two
  pairs arrive per token tile so the 4 MLP tail chains stagger into the
  stream's tail.  Consts go on the gpsimd (SWDGE) queue in parallel.
  Every DMA source is per-partition contiguous (128 descriptors).
- MLP tail runs in bf16 (y, y2, W2/64, Wq, Wv): fewer const bytes, 2x
  16-bit DVE rate.  End-to-end error ~6e-3 vs the 2e-2 gate.
- Softmax without an on-chip max pass: the host folds
  c_h = max(0, pos_w_h*(S-1)) into the additive logit table (the
  remaining y2@Wq part is O(10), safe in fp32 exp), added into the q psum
  via a small identity matmul off the critical path.  bv folds into the
  host merge: sum p*(v+bv) = W/Z + bv.  The exp/mul/reduce stats chain is
  split into column halves so ACT and DVE pipeline.

NOTE: tensor_tensor_reduce hangs the device (NRT_EXEC_UNIT_UNRECOVERABLE);
use separate tensor_mul + tensor_reduce.
"""

import os

import numpy as np

# Problem dims (hardcoded per harness contract).
B, S, H = 4, 4096, 4096
MLP, NH = 128, 8
NCORES = 8
S_SHARD = S // NCORES        # 512 seq positions per core
TOK = B * S_SHARD            # 2048 tokens per core
NT = TOK // 512              # 4 token tiles of 512 (= one batch each)
KCH = H // 128               # 32 contraction chunks of 128
NPAIR = KCH // 2             # 16 DoubleRow pairs
NTAILP = 1                   # pairs delivered per-token-tile at the end:
# with a single tail pair, its weight load is shared across the 4 tile
# matmuls AND each matmul gates only on its own tile's 128 KB transfer,
# so the tile stops stagger with DMA arrival (~0.36 us apart).
NFULLP = NPAIR - NTAILP      # 15 pairs delivered full-width
# full-group sizes in pairs: small first groups (fast ramp) and small last
# groups (small end backlog)
GSCHED = [1, 1, 2, 2, 2, 2, 2, 2, 1]
assert sum(GSCHED) == NFULLP
P32 = NT * NH                # 32 packed (tile, head) lanes

_cache = {}


def _build_nc():
    import concourse.mybir as mybir
    import concourse.tile as tile
    from concourse import bacc
    from concourse.tile import add_dep_helper

    f32 = mybir.dt.float32
    f32r = mybir.dt.float32r
    bf16 = mybir.dt.bfloat16
    fp8 = mybir.dt.float8e4

    nc = bacc.Bacc()
    # xt[p, j, i, n] = x_shardT[128*(2j+i)+p, n]
    xt_d = nc.dram_tensor("xt", [128, NFULLP, 2, TOK], fp8,
                          kind="ExternalInput")
    # tail pairs, per token tile: xtl[p, t, j, i, n]
    xtl_d = nc.dram_tensor("xtl", [128, NT, NTAILP, 2, 512], fp8,
                           kind="ExternalInput")
    w1_d = nc.dram_tensor("w1s", [128, KCH, MLP], fp8, kind="ExternalInput")
    # cwb (bf16): [w2t/64 | wq32 (4 x 32-wide zero-padded blocks) | wv32]
    CQ = MLP
    CV = MLP + P32 * NT
    CWBW = MLP + 2 * P32 * NT
    cwb_d = nc.dram_tensor("cwb", [MLP, CWBW], bf16, kind="ExternalInput")
    ci_d = nc.dram_tensor("ci", [P32, P32], f32r, kind="ExternalInput")
    cb_d = nc.dram_tensor("cb", [MLP, 2], f32, kind="ExternalInput")  # 64b1|b2
    # ca row 8t+h = (batch tile t, head h): pos*pos_w - c_h + mask add
    ca_d = nc.dram_tensor("ca", [P32, 512], f32r, kind="ExternalInput")
    # stats: [Z_lo, W_lo, Z_hi, W_hi]
    st_d = nc.dram_tensor("stats", [P32, 4], f32, kind="ExternalOutput")

    AF = mybir.ActivationFunctionType
    AX = mybir.AxisListType
    OP = mybir.AluOpType
    PM = mybir.MatmulPerfMode.DoubleRow

    with tile.TileContext(nc) as tc:
        with (
            tc.tile_pool(name="const", bufs=1) as const,
            tc.tile_pool(name="xp", bufs=len(GSCHED)) as xp,
            tc.tile_pool(name="xlp", bufs=NT) as xlp,
            tc.tile_pool(name="yp", bufs=4) as yp,
            tc.tile_pool(name="y2p", bufs=4) as y2p,
            tc.tile_pool(name="smallp", bufs=1) as smallp,
            tc.tile_pool(name="statsp", bufs=1) as statsp,
            tc.tile_pool(name="ps_y", bufs=4, space="PSUM") as ps_y,
            tc.tile_pool(name="ps_w", bufs=1, space="PSUM") as ps_w,
            tc.tile_pool(name="ps_q", bufs=1, space="PSUM") as ps_q,
            tc.tile_pool(name="ps_v", bufs=1, space="PSUM") as ps_v,
        ):
            w1_sb = const.tile([128, KCH, MLP], fp8)
            cwb_sb = const.tile([MLP, CWBW], bf16)
            ci_sb = const.tile([P32, P32], f32r)
            cb_sb = const.tile([MLP, 2], f32)
            ca_sb = const.tile([P32, 512], f32r)

            x_sb = []
            pbase = []
            p0 = 0
            for gi, gsz in enumerate(GSCHED):
                xg = xp.tile([128, gsz, 2, TOK], fp8, tag="x", name=f"x{gi}")
                x_sb.append(xg)
                pbase.append(p0)
                p0 += gsz
            xl_sb = [xlp.tile([128, NTAILP, 2, 512], fp8, tag="xl",
                              name=f"xl{t}") for t in range(NT)]

            # sync queue: first-pair weights (they gate the PE's first real
            # matmul -- on the gpsimd queue they starve against the saturated
            # sync stream and arrive ~2.4 us late), then ALL x transfers in
            # consumption order.  gpsimd queue: the other consts, in parallel.
            nc.sync.dma_start(out=w1_sb[:, 0:4, :], in_=w1_d[:, 0:4, :])
            for gi, gsz in enumerate(GSCHED):
                nc.sync.dma_start(
                    out=x_sb[gi][:],
                    in_=xt_d[:, pbase[gi]:pbase[gi] + gsz, :, :])
                if gi == 0:
                    nc.gpsimd.dma_start(out=w1_sb[:, 4:KCH, :],
                                        in_=w1_d[:, 4:KCH, :])
                    nc.gpsimd.dma_start(out=cwb_sb[:], in_=cwb_d[:])
                    nc.gpsimd.dma_start(out=ci_sb[:], in_=ci_d[:])
                    nc.gpsimd.dma_start(out=cb_sb[:], in_=cb_d[:])
                    nc.gpsimd.dma_start(out=ca_sb[:], in_=ca_d[:])
            for t in range(NT):
                nc.sync.dma_start(out=xl_sb[t][:], in_=xtl_d[:, t, :, :, :])

            stats_sb = statsp.tile([P32, 4], f32)

            # HAM warm-up: the PE sits idle through the ~12 us preamble +
            # first-DMA ramp, so HAM throttles its clock to 1.2 GHz and the
            # first ~10 real matmuls run at half speed (deferring ~2 us of
            # PE work past the stream's end).  Chew through dummy matmuls on
            # a memset tile during the ramp so the real stream starts warm.
            warm_ps = ps_w.tile([128, 512], f32, tag="warm", name="warm_ps")
            dw_sb = const.tile([128, 256], bf16)
            nc.vector.memset(dw_sb[:], 0.0)

            def pe_dummies(n):
                for _ in range(n):
                    nc.tensor.matmul(warm_ps[:, 0:256], dw_sb[:, 0:128],
                                     dw_sb[:, 0:256], start=True, stop=True)

            pe_dummies(16)

            # Warmup: PE observes the w1 first-group DMA lane before the real
            # matmuls so steady-state instructions carry at most one new wait.
            warm_pe = nc.tensor.matmul(warm_ps[0:32, 0:64],
                                       w1_sb[:, 0, 0:32], w1_sb[:, 0, 0:64],
                                       start=True, stop=True)
            warm_act = const.tile([MLP, 1], f32)
            nc.scalar.copy(out=warm_act[:], in_=cb_sb[:, 1:2])
            warm_dve = const.tile([MLP, 1], f32)
            nc.vector.tensor_copy(out=warm_dve[:], in_=cb_sb[:, 0:1])

            # Layer 1, k-major: yT[t] (128, 512) += (64*W1T)_pair.T @ x_pair,
            # DoubleRow fp8, accumulated over the 16 pairs.
            psum_y = [ps_y.tile([128, 512], f32, tag="y", name=f"y_ps{t}")
                      for t in range(NT)]
            for gi, gsz in enumerate(GSCHED):
                for jj in range(gsz):
                    jp = pbase[gi] + jj
                    for t in range(NT):
                        mm = nc.tensor.matmul(
                            psum_y[t][:],
                            w1_sb[:, 2 * jp:2 * jp + 2, :],
                            x_sb[gi][:, jj, :, t * 512:(t + 1) * 512],
                            start=(jp == 0), stop=False,
                            perf_mode=PM)
                        if jp == 0 and t == 0:
                            add_dep_helper(mm.ins, warm_pe.ins, sync=False,
                                           reason="warmup before first mm")

            # per-tile tail pairs: arrival order staggers the tile stops so
            # the DVE relu cascade starts as each tile's data lands
            for t in range(NT):
                for j in range(NTAILP):
                    jp = NFULLP + j
                    nc.tensor.matmul(psum_y[t][:],
                                     w1_sb[:, 2 * jp:2 * jp + 2, :],
                                     xl_sb[t][:, j, :, :],
                                     start=False, stop=(jp == NPAIR - 1),
                                     perf_mode=PM)

            q32_ps = ps_q.tile([P32, 512], f32, tag="q", name="q32_ps")
            v32_ps = ps_v.tile([P32, 512], f32, tag="v", name="v32_ps")
            # Fold the additive logit table into the q psum via an identity
            # matmul -- runs as soon as ca lands, off the critical tail path.
            nc.tensor.matmul(q32_ps[:], ci_sb[:], ca_sb[:],
                             start=True, stop=False)
            # Two-phase tail with DVE/ACT braiding: phase A drains each y
            # psum through relu1 (alternating engines) into its W2 matmul --
            # the y2 psum REUSES the y bank its relu1 just freed.  Phase B
            # runs the relu2s (alternating the other way) and the head
            # projections.  Emitting all relu1s before all relu2s keeps the
            # strict-FIFO ACT/DVE queues free of head-of-line blocking.
            y2_pss = []
            y2_sbs = []
            for t in range(NT):
                y_sb = yp.tile([128, 512], bf16, tag="ysb", name=f"y_sb{t}")
                # y_sb = relu(psum + 64*b1) = 64*y; the 1/64 is folded into
                # cwb's W2 block.
                if t % 2 == 0:
                    nc.vector.tensor_scalar(out=y_sb[:], in0=psum_y[t][:],
                                            scalar1=cb_sb[:, 0:1],
                                            scalar2=0.0, op0=OP.add,
                                            op1=OP.max)
                else:
                    nc.scalar.activation(out=y_sb[:], in_=psum_y[t][:],
                                         func=AF.Relu, bias=cb_sb[:, 0:1],
                                         scale=1.0)
                y2_ps = ps_y.tile([128, 512], f32, tag="y",
                                  name=f"y2_ps{t}")
                nc.tensor.matmul(y2_ps[:], cwb_sb[:, 0:MLP], y_sb[:],
                                 start=True, stop=True)
                y2_pss.append(y2_ps)
            for t in range(NT):
                y2_sb = y2p.tile([128, 512], bf16, tag="y2sb",
                                 name=f"y2_sb{t}")
                if t % 2 == 0:
                    nc.scalar.activation(out=y2_sb[:], in_=y2_pss[t][:],
                                         func=AF.Relu, bias=cb_sb[:, 1:2],
                                         scale=1.0)
                else:
                    nc.vector.tensor_scalar(out=y2_sb[:], in0=y2_pss[t][:],
                                            scalar1=cb_sb[:, 1:2],
                                            scalar2=0.0, op0=OP.add,
                                            op1=OP.max)
                y2_sbs.append(y2_sb)
                # Head projections: the (128, 32) weight block for tile t is
                # zero outside rows 8t..8t+8, so accumulating all 4 tiles into
                # one (32, 512) bank packs q/v as (tile, head) x seq lanes.
                nc.tensor.matmul(q32_ps[:],
                                 cwb_sb[:, CQ + P32 * t:CQ + P32 * (t + 1)],
                                 y2_sb[:], start=False, stop=(t == NT - 1))
                nc.tensor.matmul(v32_ps[:],
                                 cwb_sb[:, CV + P32 * t:CV + P32 * (t + 1)],
                                 y2_sb[:], start=(t == 0), stop=(t == NT - 1))

            # Stats, split into column halves so ACT (exp) and DVE
            # (mul/reduce) pipeline: Z = sum e, W = sum e*v per half.
            e_sb = smallp.tile([P32, 512], f32, tag="e", name="e_sb")
            ev_sb = smallp.tile([P32, 512], f32, tag="ev", name="ev_sb")
            for half in range(2):
                cols = slice(256 * half, 256 * (half + 1))
                nc.scalar.activation(out=e_sb[:, cols], in_=q32_ps[:, cols],
                                     func=AF.Exp, bias=0.0, scale=1.0,
                                     accum_out=stats_sb[:, 2 * half:
                                                        2 * half + 1])
                nc.vector.tensor_mul(out=ev_sb[:, cols], in0=e_sb[:, cols],
                                     in1=v32_ps[:, cols])
                nc.vector.tensor_reduce(
                    out=stats_sb[:, 2 * half + 1:2 * half + 2],
                    in_=ev_sb[:, cols], axis=AX.X, op=OP.add)

            nc.sync.dma_start(out=st_d[:], in_=stats_sb[:])

    nc.finalize()
    return nc


def get_nc():
    if "nc" not in _cache:
        _cache["nc"] = _build_nc()
    return _cache["nc"]


def make_core_inputs(x, mask, W1, b1, W2, b2, Wq, Wv, bv, pos_w, bias):
    """Host-side shard + transpose + fp8 quantize. Returns 8 in_maps."""
    import ml_dtypes
    FP8 = ml_dtypes.float8_e4m3
    BF16 = ml_dtypes.bfloat16

    # W1 scaled by 64 so its values quantize in e4m3's normal range; the
    # matching 1/64 is folded into W2 below (exact: power of two).
    w1s = np.ascontiguousarray(
        (W1.astype(np.float32) * 64.0).reshape(MLP, KCH, 128)
        .transpose(2, 1, 0)).astype(FP8)

    CQ = MLP
    CV = MLP + P32 * NT
    cwb = np.zeros((MLP, MLP + 2 * P32 * NT), dtype=np.float32)
    cwb[:, 0:MLP] = W2.T / 64.0
    for t in range(NT):
        cwb[:, CQ + P32 * t + NH * t:CQ + P32 * t + NH * (t + 1)] = Wq.T
        cwb[:, CV + P32 * t + NH * t:CV + P32 * t + NH * (t + 1)] = Wv.T
    cwb = cwb.astype(BF16)
    ci = np.eye(P32, dtype=np.float32)
    cb = np.ascontiguousarray(
        np.stack([b1.astype(np.float32) * 64.0,
                  b2.astype(np.float32)], axis=1), dtype=np.float32)

    pos = np.arange(S, dtype=np.float32)
    maskadd = np.where(mask == 0, np.float32(-1e9), np.float32(0.0))  # (B,S)
    # Host-side stability offset: dominant logit term over the FULL sequence.
    c_h = np.maximum(pos_w.astype(np.float32) * (S - 1), 0.0)       # (NH,)

    in_maps = []
    for c in range(NCORES):
        sl = slice(c * S_SHARD, (c + 1) * S_SHARD)
        xT = np.ascontiguousarray(
            x[:, sl, :].transpose(2, 0, 1).reshape(H, TOK)).astype(FP8)
        xr = xT.reshape(NPAIR, 2, 128, TOK)                 # (jp, i, p, n)
        xt8 = np.ascontiguousarray(xr[0:NFULLP].transpose(2, 0, 1, 3))
        xtl = np.ascontiguousarray(
            xr[NFULLP:].reshape(NTAILP, 2, 128, NT, 512)
            .transpose(2, 3, 0, 1, 4))              # (128, NT, 2, 2, 512)
        ca = np.empty((P32, 512), dtype=np.float32)
        add_ths = (pos_w.astype(np.float32)[None, :, None]
                   * pos[sl][None, None, :]
                   - c_h[None, :, None]
                   + maskadd[:, None, sl])            # (B=NT, NH, 512)
        ca[:, :] = add_ths.reshape(P32, 512)
        in_maps.append({"xt": xt8, "xtl": xtl, "w1s": w1s, "cwb": cwb,
                        "ci": ci, "cb": cb, "ca": ca})
    return in_maps


def merge_stats(stats_all, bv, bias):
    """stats_all: (NCORES, 32, 4), row 8t+h = (batch t, head h) with
    [Z_lo, W_lo, Z_hi, W_hi].  All cores share the same per-head logit
    offset, so the merge is a plain sum.  bv folds in on the host:
    sum_s p*(v+bv) = W/Z + bv."""
    st = np.asarray(stats_all, dtype=np.float64).reshape(NCORES, NT, NH, 4)
    Z = (st[..., 0] + st[..., 2]).sum(axis=0)        # (B, NH)
    W = (st[..., 1] + st[..., 3]).sum(axis=0)
    out = (W / Z + np.asarray(bv, dtype=np.float64)[None, :]).sum(axis=1)
    return (out[:, None] + np.float64(bias.reshape(1)[0])).astype(np.float32)


def kernel(x, mask, W1, b1, W2, b2, Wq, Wv, bv, pos_w, bias, _trace=False):
    from concourse.bass_utils import run_bass_kernel_spmd

    x = np.asarray(x, dtype=np.float32)
    in_maps = make_core_inputs(x, np.asarray(mask), *(np.asarray(a) for a in
                               (W1, b1, W2, b2, Wq, Wv, bv, pos_w, bias)))
    nc = get_nc()
    res = run_bass_kernel_spmd(nc, in_maps, core_ids=list(range(NCORES)),
                               trace=_trace)
    stats_all = np.stack([r["stats"] for r in res.results])  # (C, 32, 4)
    out = merge_stats(stats_all, np.asarray(bv), np.asarray(bias))
    if _trace:
        kernel.last_result = res
    return out
